# revision 7
# baseline (speedup 1.0000x reference)
"""Trainium2 Bass kernel for nn_Metamorph_parameterReinforcer.

Math background (exact identities, verified against the reference):
  The reference's einsum("bfp,mn->bfm", fx, wfft) sums over BOTH p and n,
  so each "STFT block" collapses:
    sum_p fft(x, norm=forward)[..., p] == x[..., 0]
    block(x)[b, f, k] = Re tanh(x[b, f, 0] * W[k]),
       W[k] = sum_m (sum_n wfft[m, n]) * exp(2j*pi*k*m/64)
  Chaining three blocks, only element 0 of the last axis propagates:
    a  = params[:, :, 0]
    s1 = Retanh(a  * W0[0]);  s2 = Retanh(s1 * W1[0])
    x3[b, f, l] = Retanh(s2[b, f] * W2[l])         # (512, 1000, 64)
    h  = tanh(x3.reshape(512, 64000) @ lin1_w.T + lin1_b)
    out = sigmoid(h @ lin2_w.T + lin2_b)
  Because |W0[0]|, |W1[0]| ~ 32000, tanh saturates and s2 is +-1 in f32 for
  all but (rare) tiny |a| entries; where saturated, x3[b, f, :] =
  s2[b, f] * x1[:] with x1 = Retanh(W2) -- rank-1.  Rare non-saturated
  entries get an exact host-side correction dH added before the lin1 tanh.

Device kernel (8 cores, lin1_w sharded over its output dim j, 125 rows/core).
The 256 MB lin1_w read is the memory roofline; it is shipped as fp8-e4m3
(8 MB/core) with x1 and a x512 scale folded in on the host.  Quantization
uses error feedback along l: the device sums all 64 l-slices of each
(j, f) group, so carrying the rounding residual into the next slice makes
the SUM accurate to one final rounding step (end-to-end rel err ~2.6e-3
vs ~1.7e-2 for independent rounding).

Per core, with f = 125*g + fp (g in 0..8, fp in 0..125):
  DRAM w1q[fp, (g, j), l] = q8(lin1_w[j, 64 f + l] * x1[l] * 512)
  stage 1 (TensorE): psum[fp, (g, j)] += w1q[:, :, l] for all l, done as
      32 fp8 DoubleRow matmuls per 250-column chunk (identity lhsT packs
      two l-slices per instruction at 0.5 cycles/column).  PSUM output is
      125 partitions wide, so evacuation to SBUF (bf16) is cheap and the
      result lands directly in stage-2 (K = fp) orientation -- no scatter.
  stage 2 (TensorE): ph[j, b] += A_g[fp, j].T @ s2g[fp, b] per g (s2 is
      pre-scaled by 1/512 on host); interleaved with stage-1 chunks.
  ScalarE: h = tanh(ph + bias) -> bf16
  stage 3: partial[k, b] = lin2_w-shard.T @ h; DMA out (f32).
Host combines the 8 partials: out = sigmoid(sum_c partial_c + lin2_b).
"""

import numpy as np

B, MODES, L = 512, 1000, 64
NCORES = 8
JSH = MODES // NCORES          # 125 lin1 output rows per core
NG = 8                         # f = 125*g + fp
FP = 125
NCHK = 4                       # gj chunks per core
CH = MODES // NCHK             # 250 (g,j) columns per chunk (16 KB DMA lines)
GPC = CH // FP                 # 2 g per chunk
NSUB = 4                       # sub-DMAs per chunk (512 KB each, 4 KB lines)
LSUB = L // NSUB               # 16 l-planes per sub-DMA
SCALE = 512.0                  # fp8 scale, folded out via s2 on host
SAT = 50.0                     # |2*s*Re(W)| beyond this: Retanh == sign


def _retanh(s, w):
    """Re tanh(s * w) for real array s and complex (array or scalar) w."""
    s = np.asarray(s, np.float64)
    x = 2.0 * np.multiply.outer(s, np.real(w))
    y = 2.0 * np.multiply.outer(s, np.imag(w))
    xc = np.clip(x, -SAT, SAT)
    with np.errstate(over="ignore", invalid="ignore"):
        r = np.sinh(xc) / (np.cosh(xc) + np.cos(y))
    return np.where(np.abs(x) >= SAT, np.sign(x), r)


def _wvec(wre, wim):
    """W[k] = sum_m (sum_n w[m, n]) * exp(2j pi k m / L)."""
    wsum = wre.astype(np.float64).sum(axis=1) + 1j * wim.astype(np.float64).sum(axis=1)
    tw = np.exp(2j * np.pi * np.outer(np.arange(L), np.arange(L)) / L)
    return tw @ wsum


_CACHE = {}


def _build_program(use_dh):
    """Build (and cache) the Bass program. Same program for all 8 cores."""
    key = ("prog", use_dh, "fp8dr_v6", NCHK)
    if key in _CACHE:
        return _CACHE[key]

    import concourse.bacc as bacc
    import concourse.mybir as mybir
    import concourse.tile as tile

    f32 = mybir.dt.float32
    bf16 = mybir.dt.bfloat16
    fp8 = mybir.dt.float8e4
    nc = bacc.Bacc("TRN2", target_bir_lowering=False, debug=False)

    f16 = mybir.dt.float16
    w1q_d = nc.dram_tensor("w1q", [FP, NCHK * L, CH], fp8, kind="ExternalInput")
    s2t_d = nc.dram_tensor("s2t", [FP, NG, B], bf16, kind="ExternalInput")
    id2_d = nc.dram_tensor("id2", [FP, 2, 128], fp8, kind="ExternalInput")
    bias_d = nc.dram_tensor("bias", [JSH, 1], f32, kind="ExternalInput")
    l2t_d = nc.dram_tensor("l2t", [JSH, L], bf16, kind="ExternalInput")
    if use_dh:
        dht_d = nc.dram_tensor("dht", [JSH, B], f32, kind="ExternalInput")
    outp_d = nc.dram_tensor("outp", [L, B], f16, kind="ExternalOutput")

    with tile.TileContext(nc) as tc:
        with (
            tc.tile_pool(name="const", bufs=1) as const,
            tc.tile_pool(name="w1pool", bufs=8) as w1pool,
            tc.tile_pool(name="acc", bufs=1) as acc,
            tc.tile_pool(name="psC", bufs=2, space="PSUM") as psC,
            tc.tile_pool(name="psH", bufs=1, space="PSUM") as psH,
            tc.tile_pool(name="psO", bufs=1, space="PSUM") as psO,
        ):
            # hand-balanced DMA schedule (engines feed disjoint HW queue
            # groups at ~80 GB/s each; keep the three streams equal and the
            # chunk-completion order monotone):
            #   sync:   c0s0 c0s3 c1s2 c2s1 c3s0 c3s3            (3.0 MB)
            #   scalar: id2  c0s1 c1s0 c1s3 c2s2 c3s1  tail-consts (2.6 MB)
            #   gpsimd: c0s2 c1s1 c2s0 c2s3 c3s2 s2g             (3.0 MB)
            eng_of = {
                (0, 0): 0, (0, 3): 0, (1, 2): 0, (2, 1): 0, (3, 0): 0, (3, 3): 0,
                (0, 1): 1, (1, 0): 1, (1, 3): 1, (2, 2): 1, (3, 1): 1,
                (0, 2): 2, (1, 1): 2, (2, 0): 2, (2, 3): 2, (3, 2): 2,
            }
            rot = [nc.sync, nc.scalar, nc.gpsimd]

            id2 = const.tile([FP, 2, 128], fp8)
            nc.scalar.dma_start(id2[:], id2_d.ap())
            s2g = const.tile([FP, NG, B], bf16)

            a_sb = acc.tile([FP, MODES], bf16)
            ph = psH.tile([JSH, B], f32)
            h_sb = acc.tile([JSH, B], bf16)
            po = psO.tile([L, B], f32)
            o_sb = acc.tile([L, B], f16)

            tiles = {}
            for c in range(NCHK):
                for s in range(NSUB):
                    t = w1pool.tile([FP, LSUB, CH], fp8, tag="w1s")
                    l0 = c * L + s * LSUB
                    rot[eng_of[(c, s)]].dma_start(
                        t[:], w1q_d.ap()[:, l0 : l0 + LSUB, :]
                    )
                    tiles[(c, s)] = t
                pc = psC.tile([128, CH], f32, tag="pc")
                for l2 in range(L // 2):
                    t = tiles[(c, l2 // (LSUB // 2))]
                    li = (l2 % (LSUB // 2)) * 2
                    nc.tensor.matmul(
                        pc[:, :],
                        id2[:],
                        t[:, li : li + 2, :],
                        start=(l2 == 0),
                        stop=(l2 == L // 2 - 1),
                        perf_mode=mybir.MatmulPerfMode.DoubleRow,
                    )
                nc.vector.tensor_copy(a_sb[:, c * CH : (c + 1) * CH], pc[0:FP, :])

            # s2g + tail-only constants: issued last so the w1 stream is
            # never delayed; they land before the stage-2/3 tail needs them.
            nc.gpsimd.dma_start(s2g[:], s2t_d.ap())
            bias = const.tile([JSH, 1], f32)
            nc.scalar.dma_start(bias[:], bias_d.ap())
            l2t = const.tile([JSH, L], bf16)
            nc.scalar.dma_start(l2t[:], l2t_d.ap())
            if use_dh:
                dht = const.tile([JSH, B], f32)
                nc.scalar.dma_start(dht[:], dht_d.ap())

            for g in range(NG):
                nc.tensor.matmul(
                    ph[:, :],
                    a_sb[:, g * FP : (g + 1) * FP],
                    s2g[:, g, :],
                    start=(g == 0),
                    stop=(g == NG - 1),
                )
            if use_dh:
                nc.vector.tensor_add(ph[:, :], ph[:, :], dht[:, :])
            nc.scalar.activation(
                h_sb[:, :],
                ph[:, :],
                mybir.ActivationFunctionType.Tanh,
                bias=bias[:, 0:1],
            )

            nc.tensor.matmul(po[:, :], l2t[:, :], h_sb[:, :], start=True, stop=True)
            nc.vector.tensor_copy(o_sb[:, :], po[:, :])
            nc.sync.dma_start(outp_d.ap(), o_sb[:, :])

    nc.compile()
    _CACHE[key] = nc
    return nc


def profile_last(trace_cores=None):
    """Re-run the last-built program with NTFF tracing (dev/test helper)."""
    if "last_run" not in _CACHE:
        return None
    from concourse.bass_utils import run_bass_kernel_spmd

    nc, in_maps = _CACHE["last_run"]
    return run_bass_kernel_spmd(
        nc,
        in_maps,
        list(range(NCORES)),
        trace=True,
        trace_cores=trace_cores,
    )


def kernel(
    params,
    wfft0_re,
    wfft0_im,
    wfft1_re,
    wfft1_im,
    wfft2_re,
    wfft2_im,
    lin1_w,
    lin1_b,
    lin2_w,
    lin2_b,
):
    from concourse.bass_utils import run_bass_kernel_spmd
    import ml_dtypes

    bf16 = ml_dtypes.bfloat16
    fp8 = ml_dtypes.float8_e4m3

    # ---- host: closed-form collapse of the three spectral blocks ----
    a = params[:, :, 0].astype(np.float64)
    w0 = _wvec(wfft0_re, wfft0_im)[0]
    w1v = _wvec(wfft1_re, wfft1_im)[0]
    w2 = _wvec(wfft2_re, wfft2_im)
    s1 = _retanh(a, w0)
    s2 = _retanh(s1, w1v).astype(np.float32)
    x1 = _retanh(np.float64(1.0), w2)  # (64,) f64

    # exact correction for entries where tanh did not saturate to +-1
    bad_b, bad_f = np.nonzero(np.abs(s2) != np.float32(1.0))
    use_dh = bad_b.size > 0
    dh = None
    if use_dh:
        dh = np.zeros((B, MODES), np.float64)
        x1_64 = x1.astype(np.float64)
        for b, f in zip(bad_b.tolist(), bad_f.tolist()):
            s = np.float64(s2[b, f])
            delta = _retanh(s, w2)[0] - s * x1_64
            dh[b, :] += lin1_w[:, 64 * f : 64 * (f + 1)].astype(np.float64) @ delta
        dh = dh.astype(np.float32)

    # ---- host: fold x1 and SCALE into lin1_w, error-feedback fp8 quant ----
    # v[j, f, l] = lin1_w[j, 64 f + l] * x1[l] * SCALE; quantize along l with
    # carry so that sum_l q[j, f, l] tracks sum_l v[j, f, l] to one step.
    v = lin1_w.reshape(MODES, MODES, L) * (
        x1.astype(np.float32) * np.float32(SCALE)
    )
    q = np.empty((MODES, MODES, L), dtype=fp8)
    carry = np.zeros((MODES, MODES), np.float32)
    for l in range(L):
        t = v[:, :, l] + carry
        ql = t.astype(fp8)
        q[:, :, l] = ql
        carry = t - ql.astype(np.float32)

    # s2t[fp, g, b] = s2[b, 125 g + fp] / SCALE  (exact in bf16 for s2 = +-1)
    s2t = np.ascontiguousarray(
        (s2.T.reshape(NG, FP, B).transpose(1, 0, 2) / np.float32(SCALE)).astype(bf16)
    )

    id2 = np.zeros((FP, 2, 128), dtype=fp8)
    idx = np.arange(FP)
    id2[idx, 0, idx] = 1.0
    id2[idx, 1, idx] = 1.0

    in_maps = []
    for c in range(NCORES):
        j0, j1 = JSH * c, JSH * (c + 1)
        # w1q[fp, (c, l), j'] = q[j0 + j, 125 g + fp, l], gj = 250 c + j'
        w1gj = q[j0:j1].reshape(JSH, NG, FP, L).transpose(2, 1, 0, 3).reshape(
            FP, MODES, L
        )
        w1q = np.ascontiguousarray(
            w1gj.reshape(FP, NCHK, CH, L).transpose(0, 1, 3, 2).reshape(
                FP, NCHK * L, CH
            )
        )
        m = {
            "w1q": w1q,
            "s2t": s2t,
            "id2": id2,
            "bias": np.ascontiguousarray(lin1_b[j0:j1].reshape(JSH, 1)),
            "l2t": np.ascontiguousarray(lin2_w[:, j0:j1].T.astype(bf16)),
        }
        if use_dh:
            m["dht"] = np.ascontiguousarray(dh[:, j0:j1].T)
        in_maps.append(m)

    nc = _build_program(use_dh)
    _CACHE["last_run"] = (nc, in_maps)
    res = run_bass_kernel_spmd(nc, in_maps, list(range(NCORES)))

    acc = np.zeros((L, B), np.float64)
    for c in range(NCORES):
        acc += res.results[c]["outp"].astype(np.float64)
    out = 1.0 / (1.0 + np.exp(-(acc.T + lin2_b.astype(np.float64))))
    return out.astype(np.float32)
